# revision 3
# baseline (speedup 1.0000x reference)
"""Pairwise cosine-similarity kernel for Trainium2 (8 NeuronCores, SPMD).

Computes out = 16 * normalize(x1) @ normalize(x2).T for x1, x2 [8192, 512] f32.

Sharding: x1 rows are split across the 8 cores (1024 rows each); x2 is
replicated. Each core computes its [1024, 8192] slice of the output; the host
concatenates the slices.

Per-core pipeline (all compute on device):
  1. Load x1 block + x2 (f32), compute per-row sum-of-squares with a fused
     Square+row-reduce on the scalar engine, then 1/max(sqrt(ss), eps) (DVE).
  2. Transpose x1/x2 into [D, rows] layout with PE matmuls against a
     per-row-scaled diagonal matrix: out = tile.T @ diag(inv_norm) transposes
     AND normalizes in one pass (x1's diagonal also folds in the 16x scale).
     PSUM results are copied to SBUF with a cast to bf16.
  3. Main GEMM: out_tile[128, 512] += x1nT.T @ x2nT over 4 K-chunks (bf16
     inputs, f32 PSUM accumulate), PSUM->SBUF copy split across DVE/ACT,
     then DMA to DRAM.
"""

import sys

for _p in ("/root/.axon_site/_ro/trn_rl_repo", "/opt/trn_rl_repo"):
    if _p not in sys.path:
        sys.path.append(_p)

import numpy as np

import concourse.bass as bass
import concourse.tile as tile
from concourse import bacc, mybir
from concourse.bass_utils import run_bass_kernel_spmd
from concourse.masks import make_identity

F32 = mybir.dt.float32
P = 128
SCALE = 16.0
EPS = 1e-8

N_CORES = 8
N1 = 8192  # x1 rows (total)
N2 = 8192  # x2 rows
D = 512  # feature dim

_PROGRAM_CACHE = {}


def build_program(
    n1_local=N1 // N_CORES,
    n2=N2,
    d=D,
    mm_dtype=mybir.dt.bfloat16,
    cg_width=2048,
):
    """Build the SPMD program one core runs. Returns the compiled Bacc."""
    kc = d // P  # K-chunks of the contraction dim
    m_tiles = n1_local // P  # x1 row-tiles per core
    n_cgs = n2 // cg_width  # output column groups
    g_rows = 512  # x2/x1 rows prepped per group (4 row-tiles)
    nch = cg_width // 512  # 512-wide output chunks per column group

    nc = bacc.Bacc("TRN2", target_bir_lowering=False, debug=False,
                   num_devices=N_CORES)
    x1 = nc.dram_tensor("x1", [n1_local, d], F32, kind="ExternalInput")
    x2 = nc.dram_tensor("x2", [n2, d], F32, kind="ExternalInput")
    out = nc.dram_tensor("out", [n1_local, n2], F32, kind="ExternalOutput")

    with tile.TileContext(nc) as tc:
        with (
            tc.tile_pool(name="const", bufs=1) as const,
            tc.tile_pool(name="ld", bufs=3) as ld,
            tc.tile_pool(name="sq", bufs=3) as sqp,
            tc.tile_pool(name="stat", bufs=6) as stat,
            tc.tile_pool(name="diag", bufs=10) as diagp,
            tc.tile_pool(name="xt", bufs=1) as xt,
            tc.tile_pool(name="outp", bufs=3) as outp,
            tc.tile_pool(name="pst", bufs=2, space="PSUM") as pst,
            tc.tile_pool(name="ps", bufs=6, space="PSUM") as psp,
        ):
            identity = const.tile([P, P], F32)
            make_identity(nc, identity)

            x1r = x1.ap().rearrange("(g j p) e -> g p j e", j=4, p=P)
            x2r = x2.ap().rearrange("(g j p) e -> g p j e", j=4, p=P)

            # Persistent transposed-normalized operands (bf16).
            x1T = [xt.tile([P, n1_local], mm_dtype, tag=f"x1T_{k}", name=f"x1T_{k}")
                   for k in range(kc)]
            x2T = [
                [xt.tile([P, cg_width], mm_dtype, tag=f"x2T_{k}_{cg}", name=f"x2T_{k}_{cg}")
                 for cg in range(n_cgs)]
                for k in range(kc)
            ]

            def prep_group(src_r, g, dst_tiles, dst_col, scale_const):
                """Normalize+transpose 4 row-tiles (512 rows) of src.

                Writes [P, 512] bf16 into dst_tiles[k][:, dst_col:dst_col+512]
                for each K-chunk k.
                """
                ld_t = ld.tile([P, 4, d], F32, tag="ld")
                nc.sync.dma_start(ld_t[:], src_r[g])
                ssq = stat.tile([P, 4], F32, tag="ssq")
                for j in range(4):
                    sq_t = sqp.tile([P, d], F32, tag="sq")
                    nc.scalar.activation(
                        sq_t[:], ld_t[:, j],
                        mybir.ActivationFunctionType.Square,
                        accum_out=ssq[:, j : j + 1],
                    )
                nrm = stat.tile([P, 4], F32, tag="nrm")
                nc.scalar.activation(
                    nrm[:], ssq[:], mybir.ActivationFunctionType.Sqrt
                )
                nc.vector.tensor_scalar_max(nrm[:], nrm[:], EPS)
                rin = stat.tile([P, 4], F32, tag="rin")
                nc.vector.reciprocal(rin[:], nrm[:])
                if scale_const != 1.0:
                    nc.vector.tensor_scalar_mul(rin[:], rin[:], scale_const)
                diags = []
                for j in range(4):
                    dg = diagp.tile([P, P], F32, tag="dg")
                    nc.vector.tensor_mul(
                        dg[:], identity[:],
                        rin[:, j : j + 1].to_broadcast((P, P)),
                    )
                    diags.append(dg)
                for k in range(kc):
                    ps_t = pst.tile([P, 512], F32, tag="pst")
                    for j in range(4):
                        # ps[dchunk, row] = tile[row, dchunk] * inv_norm[row]
                        nc.tensor.matmul(
                            ps_t[:, j * P : (j + 1) * P],
                            lhsT=ld_t[:, j, k * P : (k + 1) * P],
                            rhs=diags[j][:],
                            start=True, stop=True,
                        )
                    dst = dst_tiles[k][:, dst_col : dst_col + 512]
                    if k % 2 == 0:
                        nc.vector.tensor_copy(dst, ps_t[:])
                    else:
                        nc.scalar.copy(dst, ps_t[:])

            # x1 prep (16x inv-norm folds the output scale in).
            for g in range(n1_local // g_rows):
                prep_group(x1r, g, x1T, g * g_rows, SCALE)

            # x2 prep + main GEMM, interleaved by column group so the GEMM of
            # group cg overlaps the prep of group cg+1.
            for cg in range(n_cgs):
                for g2 in range(cg_width // g_rows):
                    g = cg * (cg_width // g_rows) + g2
                    prep_group(x2r, g, [x2T[k][cg] for k in range(kc)],
                               g2 * g_rows, 1.0)
                for m in range(m_tiles):
                    pss = [psp.tile([P, 512], F32, tag="ps", name=f"ps_{cg}_{m}_{j}")
                           for j in range(nch)]
                    for k in range(kc):
                        for j in range(nch):
                            nc.tensor.matmul(
                                pss[j][:],
                                lhsT=x1T[k][:, m * P : (m + 1) * P],
                                rhs=x2T[k][cg][:, j * 512 : (j + 1) * 512],
                                start=(k == 0), stop=(k == kc - 1),
                            )
                    ot = outp.tile([P, cg_width], F32, tag="ot")
                    for j in range(nch):
                        dst = ot[:, j * 512 : (j + 1) * 512]
                        if j % 2 == 0:
                            nc.vector.tensor_copy(dst, pss[j][:])
                        else:
                            nc.scalar.copy(dst, pss[j][:])
                    nc.sync.dma_start(
                        out[m * P : (m + 1) * P,
                            cg * cg_width : (cg + 1) * cg_width],
                        ot[:],
                    )

    nc.compile()
    return nc


def _get_program():
    key = "default"
    if key not in _PROGRAM_CACHE:
        _PROGRAM_CACHE[key] = build_program()
    return _PROGRAM_CACHE[key]


def kernel(x1: np.ndarray, x2: np.ndarray) -> np.ndarray:
    x1 = np.ascontiguousarray(np.asarray(x1, dtype=np.float32))
    x2 = np.ascontiguousarray(np.asarray(x2, dtype=np.float32))
    assert x1.shape == (N1, D) and x2.shape == (N2, D), (x1.shape, x2.shape)

    nc = _get_program()
    rows = N1 // N_CORES
    in_maps = [
        {"x1": x1[c * rows : (c + 1) * rows], "x2": x2}
        for c in range(N_CORES)
    ]
    res = run_bass_kernel_spmd(nc, in_maps, core_ids=list(range(N_CORES)))
    return np.concatenate([res.results[c]["out"] for c in range(N_CORES)], axis=0)


if __name__ == "__main__":
    rng = np.random.default_rng(0)
    a = rng.standard_normal((N1, D), dtype=np.float32)
    b = rng.standard_normal((N2, D), dtype=np.float32)
    got = kernel(a, b)
    n1 = np.maximum(np.linalg.norm(a, axis=-1, keepdims=True), EPS)
    n2 = np.maximum(np.linalg.norm(b, axis=-1, keepdims=True), EPS)
    want = SCALE * (a / n1) @ (b / n2).T
    err = np.abs(got - want)
    rel = np.linalg.norm(got - want) / np.linalg.norm(want)
    print(f"max abs err: {err.max():.3e}  rel: {rel:.3e}")


# revision 7
# speedup vs baseline: 1.1280x; 1.1280x over previous
"""Pairwise cosine-similarity kernel for Trainium2 (8 NeuronCores, SPMD).

Computes out = 16 * normalize(x1) @ normalize(x2).T for x1, x2 [8192, 512] f32.

Sharding: x1 rows are split across the 8 cores (1024 rows each); x2 is
replicated. Each core computes its [1024, 8192] slice of the output; the host
concatenates the slices.

Host-side prep is layout/dtype only: inputs are cast to bf16 and x2 is
additionally shipped pre-transposed ([512, 8192]) so the big operand needs no
on-device transposition. All FLOPs (norms, normalization, GEMM, scaling) run
on device:

  1. x1 (bf16, natural): fused Square+row-sum on ScalarE -> sqrt -> clamp ->
     reciprocal -> x1n = x1 * (16/n1) via per-partition tensor_scalar, then
     PE-transpose (bf16 matmul vs. identity) into x1T [D, rows].
  2. x2 norms from the natural-layout bf16 copy (per column-group of 2048
     rows): Square+row-sum, sqrt, clamp, reciprocal -> inv2 [128, 16] compact.
     PE-transpose inv2 to [16, 128], then broadcast across partitions with
     K=1 ones-matmuls -> inv2_bcast [128, 2048] f32, and scale the
     pre-transposed x2T tiles in place (DVE tensor_tensor).
  3. Main GEMM: out_tile[128, 512] += x1T.T @ x2T over 4 K-chunks (bf16,
     f32 PSUM), PSUM->SBUF copies split across DVE/ACT, DMA out.
"""

import sys

for _p in ("/root/.axon_site/_ro/trn_rl_repo", "/opt/trn_rl_repo"):
    if _p not in sys.path:
        sys.path.append(_p)

import ml_dtypes
import numpy as np

import concourse.bass as bass
import concourse.tile as tile
from concourse import bacc, mybir
from concourse.bass_utils import run_bass_kernel_spmd
from concourse.masks import make_identity

F32 = mybir.dt.float32
BF16 = mybir.dt.bfloat16
P = 128
SCALE = 16.0
EPS = 1e-8

N_CORES = 8
N1 = 8192  # x1 rows (total)
N2 = 8192  # x2 rows
D = 512  # feature dim

_PROGRAM_CACHE = {}


def build_program(n1_local=N1 // N_CORES, n2=N2, d=D, cg_width=2048):
    """Build the SPMD program one core runs. Returns the compiled Bacc.

    DRAM inputs: x1 [n1_local, d] bf16 (natural), x2n [n2, d] bf16 (natural,
    norms only), x2t [d, n2] bf16 (pre-transposed, GEMM operand).
    """
    kc = d // P  # K-chunks of the contraction dim
    m_tiles = n1_local // P  # x1 row-tiles per core
    n_cgs = n2 // cg_width  # output column groups
    nch = cg_width // 512  # 512-wide chunks per column group
    cg_rt = cg_width // P  # x2 row-tiles per column group

    nc = bacc.Bacc("TRN2", target_bir_lowering=False, debug=False,
                   num_devices=N_CORES)
    x1 = nc.dram_tensor("x1", [n1_local, d], BF16, kind="ExternalInput")
    x2n = nc.dram_tensor("x2n", [n2, d], BF16, kind="ExternalInput")
    x2t = nc.dram_tensor("x2t", [d, n2], BF16, kind="ExternalInput")
    out = nc.dram_tensor("out", [n1_local, n2], F32, kind="ExternalOutput")

    with tile.TileContext(nc) as tc:
        with (
            tc.tile_pool(name="const", bufs=1) as const,
            tc.tile_pool(name="ld", bufs=3) as ld,
            tc.tile_pool(name="sq", bufs=3) as sqp,
            tc.tile_pool(name="stat", bufs=4) as stat,
            tc.tile_pool(name="xt", bufs=1) as xt,
            tc.tile_pool(name="bc", bufs=2) as bcp,
            tc.tile_pool(name="outp", bufs=3) as outp,
            tc.tile_pool(name="ps", bufs=8, space="PSUM") as psp,
        ):
            ident_b = const.tile([P, P], BF16)
            make_identity(nc, ident_b)
            ident_f = const.tile([P, P], F32)
            make_identity(nc, ident_f)
            ones128 = const.tile([P, P], F32)
            nc.gpsimd.memset(ones128[:], 1.0)

            x1r = x1.ap().rearrange("(g j p) e -> g p j e", j=4, p=P)
            x2r = x2n.ap().rearrange("(g j p) e -> g p j e", j=4, p=P)

            x1T = [xt.tile([P, n1_local], BF16, tag=f"x1T_{k}", name=f"x1T_{k}")
                   for k in range(kc)]
            x2T = [
                [xt.tile([P, cg_width], BF16, tag=f"x2T_{k}_{cg}",
                         name=f"x2T_{k}_{cg}")
                 for cg in range(n_cgs)]
                for k in range(kc)
            ]

            def row_stats(src_r, g, inv_dst, scale_const):
                """inv_dst [P, 4] = scale / max(row_norm, EPS) for 4 row-tiles."""
                ld_t = ld.tile([P, 4, d], BF16, tag="ld")
                nc.sync.dma_start(ld_t[:], src_r[g])
                ssq = stat.tile([P, 4], F32, tag="ssq")
                for j in range(4):
                    sq_t = sqp.tile([P, d], BF16, tag="sq")
                    nc.scalar.activation(
                        sq_t[:], ld_t[:, j],
                        mybir.ActivationFunctionType.Square,
                        accum_out=ssq[:, j : j + 1],
                    )
                nrm = stat.tile([P, 4], F32, tag="nrm")
                nc.scalar.activation(
                    nrm[:], ssq[:], mybir.ActivationFunctionType.Sqrt
                )
                nc.vector.tensor_scalar_max(nrm[:], nrm[:], EPS)
                nc.vector.reciprocal(inv_dst, nrm[:])
                if scale_const != 1.0:
                    nc.vector.tensor_scalar_mul(inv_dst, inv_dst, scale_const)
                return ld_t

            # ---- x1: stats -> normalize (bf16) -> PE transpose ----------
            for g in range(n1_local // 512):
                inv1 = stat.tile([P, 4], F32, tag="inv1")
                ld_t = row_stats(x1r, g, inv1[:], SCALE)
                x1nrm = sqp.tile([P, 4, d], BF16, tag="x1nrm")
                for j in range(4):
                    nc.vector.tensor_scalar_mul(
                        x1nrm[:, j], ld_t[:, j], inv1[:, j : j + 1]
                    )
                for k in range(kc):
                    ps_t = psp.tile([P, 512], F32, tag="ps")
                    for j in range(4):
                        nc.tensor.matmul(
                            ps_t[:, j * P : (j + 1) * P],
                            lhsT=x1nrm[:, j, k * P : (k + 1) * P],
                            rhs=ident_b[:],
                            start=True, stop=True,
                        )
                    dst = x1T[k][:, g * 512 : (g + 1) * 512]
                    if k % 2 == 0:
                        nc.vector.tensor_copy(dst, ps_t[:])
                    else:
                        nc.scalar.copy(dst, ps_t[:])

            # ---- x2 per column group: stats -> bcast -> scale -> GEMM ---
            for cg in range(n_cgs):
                # load this cg's slice of the pre-transposed operand
                for k in range(kc):
                    nc.sync.dma_start(
                        x2T[k][cg][:],
                        x2t[k * P : (k + 1) * P,
                            cg * cg_width : (cg + 1) * cg_width],
                    )
                # compact inverse norms for the cg's rows: [P, cg_rt]
                inv2 = stat.tile([P, cg_rt], F32, tag="inv2")
                for g2 in range(cg_rt // 4):
                    row_stats(x2r, cg * (cg_rt // 4) + g2,
                              inv2[:, g2 * 4 : (g2 + 1) * 4], 1.0)
                # partition-broadcast: bc[:, c*P+p] = inv2[p, c] via
                # ones128.T @ diag(inv2[:, c]) (column sums of a diagonal)
                bc = bcp.tile([P, cg_width], F32, tag="bc")
                for c0 in range(0, cg_rt, 4):
                    ps_b = psp.tile([P, 512], F32, tag="ps")
                    for c in range(c0, c0 + 4):
                        dg = stat.tile([P, P], F32, tag="dg")
                        nc.vector.tensor_mul(
                            dg[:], ident_f[:],
                            inv2[:, c : c + 1].to_broadcast((P, P)),
                        )
                        nc.tensor.matmul(
                            ps_b[:, (c - c0) * P : (c - c0 + 1) * P],
                            lhsT=ones128[:],
                            rhs=dg[:],
                            start=True, stop=True,
                        )
                    nc.vector.tensor_copy(
                        bc[:, c0 * P : (c0 + 4) * P], ps_b[:]
                    )
                # scale the transposed operand in place (bf16 * f32 -> bf16)
                for k in range(kc):
                    nc.vector.tensor_mul(
                        x2T[k][cg][:], x2T[k][cg][:], bc[:]
                    )

                # ---- main GEMM for this column group --------------------
                for m in range(m_tiles):
                    pss = [psp.tile([P, 512], F32, tag="ps",
                                    name=f"ps_{cg}_{m}_{j}")
                           for j in range(nch)]
                    for k in range(kc):
                        for j in range(nch):
                            nc.tensor.matmul(
                                pss[j][:],
                                lhsT=x1T[k][:, m * P : (m + 1) * P],
                                rhs=x2T[k][cg][:, j * 512 : (j + 1) * 512],
                                start=(k == 0), stop=(k == kc - 1),
                            )
                    ot = outp.tile([P, cg_width], F32, tag="ot")
                    for j in range(nch):
                        dst = ot[:, j * 512 : (j + 1) * 512]
                        if j % 2 == 0:
                            nc.vector.tensor_copy(dst, pss[j][:])
                        else:
                            nc.scalar.copy(dst, pss[j][:])
                    nc.sync.dma_start(
                        out[m * P : (m + 1) * P,
                            cg * cg_width : (cg + 1) * cg_width],
                        ot[:],
                    )

    nc.compile()
    return nc


def _get_program():
    key = "default"
    if key not in _PROGRAM_CACHE:
        _PROGRAM_CACHE[key] = build_program()
    return _PROGRAM_CACHE[key]


def make_in_maps(x1: np.ndarray, x2: np.ndarray) -> list:
    x1 = np.asarray(x1, dtype=np.float32)
    x2 = np.asarray(x2, dtype=np.float32)
    assert x1.shape == (N1, D) and x2.shape == (N2, D), (x1.shape, x2.shape)
    x1_b = x1.astype(ml_dtypes.bfloat16)
    x2_b = x2.astype(ml_dtypes.bfloat16)
    x2t_b = np.ascontiguousarray(x2_b.T)
    rows = N1 // N_CORES
    return [
        {
            "x1": np.ascontiguousarray(x1_b[c * rows : (c + 1) * rows]),
            "x2n": x2_b,
            "x2t": x2t_b,
        }
        for c in range(N_CORES)
    ]


def kernel(x1: np.ndarray, x2: np.ndarray) -> np.ndarray:
    nc = _get_program()
    in_maps = make_in_maps(x1, x2)
    res = run_bass_kernel_spmd(nc, in_maps, core_ids=list(range(N_CORES)))
    return np.concatenate([res.results[c]["out"] for c in range(N_CORES)], axis=0)


if __name__ == "__main__":
    rng = np.random.default_rng(0)
    a = rng.standard_normal((N1, D), dtype=np.float32)
    b = rng.standard_normal((N2, D), dtype=np.float32)
    got = kernel(a, b)
    n1 = np.maximum(np.linalg.norm(a, axis=-1, keepdims=True), EPS)
    n2 = np.maximum(np.linalg.norm(b, axis=-1, keepdims=True), EPS)
    want = SCALE * (a / n1) @ (b / n2).T
    err = np.abs(got - want)
    rel = np.linalg.norm(got - want) / np.linalg.norm(want)
    print(f"max abs err: {err.max():.3e}  rel: {rel:.3e}")
